# revision 17
# baseline (speedup 1.0000x reference)
"""Fused Attention2D (LayerNorm -> QKV -> MHA -> proj) as a Bass/Tile kernel
for 8 Trainium2 NeuronCores.

Sharding: batch*head parallel. Core c handles batch b = c//2 and heads
4*(c%2) .. 4*(c%2)+3 (4 of the 8 heads).  Each core:
  - computes LayerNorm stats of its batch's X (over channels, via
    ones-matmuls on the tensor engine; X stays in its natural (C, hw)
    layout so no transposes are needed),
  - folds LN into the QKV projection:  qkv^T = Wg^T @ (X*r) + g*(-mu*r) + c0
    (the rank-1 terms ride along as two extra contraction rows),
  - runs attention per head in the "transposed" dataflow: S^T = K^T' Q',
    E = exp(S), O^T = [V|1]^T E  (the appended ones-column yields the
    softmax denominator for free), O normalized by 1/den,
  - applies the output projection restricted to its heads' columns,
    producing a partial (C, hw) output (the two cores of a batch each
    produce a partial sum over half the channels-in / all channels-out).
Host sums the two partials per batch.
"""

import sys

if "/opt/trn_rl_repo" not in sys.path:
    sys.path.insert(0, "/opt/trn_rl_repo")

import numpy as np
import ml_dtypes

import concourse.bass as bass
import concourse.mybir as mybir
import concourse.tile as tile

from concourse.vector_clock import ScopedClock

F32 = mybir.dt.float32
F32R = mybir.dt.float32r
BF16 = mybir.dt.bfloat16
AF = mybir.ActivationFunctionType
OP = mybir.AluOpType

B, C, H, W = 4, 512, 64, 64
NH = 8
D = 64
EPS = 1e-5
LOG2E = 1.4426950408889634
LN2 = 0.6931471805599453
SM_SCALE = float(np.sqrt(C // NH))  # 8.0

P = 128           # partitions
CHUNK = 512       # position chunk for LN/QKV phase
KT_C = C // P     # 4 contraction tiles over channels
# per-core head count and pairing
NHEADS_CORE = 4
NPAIR = 2


# ---------------------------------------------------------------------------
# Custom DVE op: approximate 2^x for the softmax, so part of the exp work can
# run on the vector engine in parallel with the scalar engine's exact exp.
# u = x + (2^23+127) makes bits(u) = 0x4B000000 + 127 + round(x); shifting
# those bits left by 23 yields the float 2^round(x); the fractional part is
# corrected by 1+(1+c)^2 = 2*(1+c+c^2/2) ~ 2*2^f with c = f*C2 — the global
# factor 2 cancels in the softmax normalization. Max relative deviation after
# the cancel: ~0.26%.
EXP2_MAGIC = float(2.0 ** 23 + 127)
EXP2_C1 = 0.7065
EXP2_SHIFT23 = float(np.frombuffer(np.uint32(23).tobytes(), dtype=np.float32)[0])

_EXP2_OP_CACHE = {}


def _get_exp2_op():
    if "op" in _EXP2_OP_CACHE:
        return _EXP2_OP_CACHE["op"]
    from concourse.dve_ops import OPS, DveOp, get_dve_sub_opcode, has_src1
    from concourse.dve_spec import Spec, Src0, Src1, C0, C1, C2, One, Bin, sq, \
        AluOp, lower
    from concourse.dve_uop import DveOpSpec

    for op in OPS:
        if op.name == "EXP2_SOFTMAX_ANT":
            _EXP2_OP_CACHE["op"] = op
            return op

    def _ref(in0, in1, s0, s1, imm2):
        in0 = in0.astype(np.float32)
        in1 = in1.astype(np.float32)
        nf = (in1 - np.float32(s0)).astype(np.float32)
        f = (in0 - nf).astype(np.float32)
        c = (f * np.float32(imm2)).astype(np.float32)
        w = (c + np.float32(1.0)).astype(np.float32)
        pp = (np.float32(1.0) + w * w).astype(np.float32)
        shift = np.asarray(s1, np.float32).reshape(-1, 1).view(np.uint32)
        e2n = (in1.view(np.uint32) << shift).view(np.float32)
        return (pp * e2n).astype(np.float32)

    nf = Src1 - C0
    f = Src0 - nf
    c = f * C2
    w = c + One
    pp = One + sq(w)
    e2n = Bin(AluOp.LOGICAL_SHIFT_LEFT, Src1, C1)
    op = DveOp("EXP2_SOFTMAX_ANT", Spec(body=pp * e2n, reference=_ref),
               subdim=False, uops_sha={})
    OPS.append(op)
    for ver in ("v3", "v4"):
        compiled = DveOpSpec(name=op.name, opcode=get_dve_sub_opcode(op.name),
                             uops=lower(op.spec, ver=ver),
                             rd1_en=has_src1(op.spec))
        op.uops_sha[ver] = compiled.sha(ver)
    _EXP2_OP_CACHE["op"] = op
    return op


# Max sync-waits the walrus build in this environment accepts per
# instruction class (learned empirically; CTRL-type and DMA take 1).
_WAIT_LIMITS = {
    "InstDMACopy": 1, "InstDrain": 1, "InstNoOp": 1, "InstEventSemaphore": 1,
    "InstDMA": 1, "InstDmaTransposeAnt": 1, "InstHalt": 1,
}
_WAIT_LIMIT_DEFAULT = 1   # compute instructions


def _hoist_excess_waits(nc):
    """Split instructions' sync-waits so no instruction carries more waits
    than this walrus accepts: excess waits move to same-engine NoOps placed
    immediately before the instruction (engine streams are in-order, so the
    semantics are unchanged)."""
    nid = 0
    for fn in nc.m.functions:
        for bb in fn.blocks:
            new = []
            for inst in bb.instructions:
                si = getattr(inst, "sync_info", None)
                waits = list(si.on_wait) if si is not None and si.on_wait else []
                lim = _WAIT_LIMITS.get(type(inst).__name__, _WAIT_LIMIT_DEFAULT)
                if len(waits) > lim:
                    for w in waits[:-lim] if lim > 0 else waits:
                        nop = mybir.InstNoOp(
                            name=f"I-hoistw{nid}", engine=inst.engine,
                            ins=[], outs=[],
                            sync_info=mybir.SyncInfo(on_wait=[w], on_update=[]))
                        nid += 1
                        new.append(nop)
                    si.on_wait = waits[-lim:]
                new.append(inst)
            bb.instructions[:] = new


class SplitDrainTileContext(tile.TileContext):
    """TileContext whose kernel-tail drain splits its semaphore waits across
    nop instructions: the walrus build in this environment only accepts one
    sync-wait per TPB CTRL (Drain/NoOp) instruction."""

    def _drain_and_barrier(self, tick_clock, wait_clock):
        probe = self.nc.sync.nop(nofuse=True, hint="tail_waits")
        wait_clock.add_sem_waits(probe.ins,
                                 ScopedClock({None: tick_clock.global_clock}))
        waits = list(probe.ins.sync_info.on_wait)
        if len(waits) > 1:
            probe.ins.sync_info.on_wait = waits[:1]
            for i in range(1, len(waits)):
                n2 = self.nc.sync.nop(nofuse=True, hint="tail_waits")
                n2.ins.sync_info = mybir.SyncInfo(on_wait=[waits[i]],
                                                  on_update=[])
        self.nc.sync.drain()
        self.nc.all_engine_barrier()
        assert self.sems is not None
        popped = self.nc._tile_sem_poison_stack.pop()
        assert popped is self._sem_poison
        self.nc.clear_and_free_semaphores(list(self.sems.allocated().values()))
        self.nc.all_engine_barrier()


def build_core_program(HW=H * W, exp_dve_ratio=0, hoist=True):
    """Build the per-core Bass program (same program for all 8 cores).

    HW: number of positions (4096 full-size; smaller for simulation tests).
    """
    nchunk = HW // CHUNK
    nqc = HW // CHUNK      # q chunks of 512
    nkt = HW // P          # k tiles of 128

    nc = bass.Bass()

    x = nc.dram_tensor("x", [C, HW], F32, kind="ExternalInput")
    wt = nc.dram_tensor("wt", [KT_C + 1, P, 512], BF16, kind="ExternalInput")
    wv = nc.dram_tensor("wv", [KT_C + 1, P, 260], BF16, kind="ExternalInput")
    wp = nc.dram_tensor("wp", [NHEADS_CORE, D, 512], BF16, kind="ExternalInput")
    bp = nc.dram_tensor("bp", [C], F32, kind="ExternalInput")
    out = nc.dram_tensor("out", [C, HW], F32, kind="ExternalOutput")

    from contextlib import ExitStack
    with SplitDrainTileContext(nc) as tc, ExitStack() as ctx:
        singles = ctx.enter_context(tc.tile_pool(name="singles", bufs=1))
        xpool = ctx.enter_context(tc.tile_pool(name="xpool", bufs=2))
        xscp = ctx.enter_context(tc.tile_pool(name="xscp", bufs=2))
        statp = ctx.enter_context(tc.tile_pool(name="statp", bufs=2))
        rbp = ctx.enter_context(tc.tile_pool(name="rbp", bufs=2))
        epool = ctx.enter_context(tc.tile_pool(name="epool", bufs=4))
        onp = ctx.enter_context(tc.tile_pool(name="onp", bufs=8))
        outp = ctx.enter_context(tc.tile_pool(name="outp", bufs=2))
        rdp = ctx.enter_context(tc.tile_pool(name="rdp", bufs=4))
        drp = ctx.enter_context(tc.tile_pool(name="drp", bufs=4, space="DRAM"))
        ps_big = ctx.enter_context(tc.tile_pool(name="ps_big", bufs=2, space="PSUM"))
        ps_small = ctx.enter_context(tc.tile_pool(name="ps_small", bufs=4, space="PSUM"))

        # ---- load weights (once) ----
        wt_sb = singles.tile([P, KT_C + 1, 512], BF16)
        nc.sync.dma_start(out=wt_sb, in_=wt[:].rearrange("k p j -> p k j"))
        wv_sb = singles.tile([P, KT_C + 1, 260], BF16)
        nc.sync.dma_start(out=wv_sb, in_=wv[:].rearrange("k p j -> p k j"))
        wp_sb = singles.tile([D, NHEADS_CORE, 512], BF16)
        nc.sync.dma_start(out=wp_sb, in_=wp[:].rearrange("h d o -> d h o"))
        bp_sb = singles.tile([P, C // P], F32)
        nc.sync.dma_start(out=bp_sb, in_=bp[:].rearrange("(oc p) -> p oc", p=P))
        ones_c = singles.tile([P, 1], BF16)
        nc.vector.memset(ones_c, 1.0)
        eps_t = singles.tile([1, 1], F32)
        nc.vector.memset(eps_t, EPS)
        ones_row = singles.tile([1, CHUNK], BF16)
        nc.vector.memset(ones_row, 1.0)

        # persistent Q/K (head-pair stacked, latent-major) and V (+ones col)
        qk_tiles = [singles.tile([P, HW], BF16, tag=f"qk{i}", name=f"qk{i}")
                    for i in range(4)]
        # order: 0: Q pair0, 1: K pair0, 2: Q pair1, 3: K pair1
        v_tiles = [singles.tile([P, nkt, 130], BF16, tag=f"v{i}", name=f"v{i}")
                   for i in range(NPAIR)]

        x_re = x[:].rearrange("(kt p) n -> p kt n", p=P)

        # ================= Phase A+B: LN stats + QKV + V =================
        for ch in range(nchunk):
            cs = ch * CHUNK
            xc = xpool.tile([P, KT_C, CHUNK], F32, tag="xc")
            nc.sync.dma_start(out=xc, in_=x_re[:, :, cs:cs + CHUNK])
            xbf = xpool.tile([P, KT_C, CHUNK], BF16, tag="xbf")
            nc.vector.tensor_copy(out=xbf, in_=xc)
            xsq = xpool.tile([P, KT_C, CHUNK], BF16, tag="xsq")
            nc.vector.tensor_tensor(xsq, xc, xc, OP.mult)

            psx = ps_small.tile([1, CHUNK], F32, tag="small")
            psq = ps_small.tile([1, CHUNK], F32, tag="small")
            for kt in range(KT_C):
                nc.tensor.matmul(psx, ones_c, xbf[:, kt],
                                 start=(kt == 0), stop=(kt == KT_C - 1))
            for kt in range(KT_C):
                nc.tensor.matmul(psq, ones_c, xsq[:, kt],
                                 start=(kt == 0), stop=(kt == KT_C - 1))

            mu = statp.tile([1, CHUNK], F32, tag="mu")
            nc.vector.tensor_scalar_mul(mu, psx, 1.0 / C)
            ex2 = statp.tile([1, CHUNK], F32, tag="ex2")
            nc.vector.tensor_scalar_mul(ex2, psq, 1.0 / C)
            varr = statp.tile([1, CHUNK], F32, tag="varr")
            nc.vector.tensor_tensor(varr, mu, mu, OP.mult)
            nc.vector.tensor_tensor(varr, ex2, varr, OP.subtract)
            lnv = statp.tile([1, CHUNK], F32, tag="lnv")
            nc.scalar.activation(lnv, varr, AF.Ln, bias=eps_t[:], scale=1.0)
            rr = statp.tile([1, CHUNK], F32, tag="rr")
            nc.scalar.activation(rr, lnv, AF.Exp, scale=-0.5)
            murb = statp.tile([1, CHUNK], F32, tag="murb")
            nc.vector.scalar_tensor_tensor(murb, mu, -1.0, rr,
                                           op0=OP.mult, op1=OP.mult)

            # broadcast r across partitions via DMA, then scale X -> bf16
            r_b = rbp.tile([P, CHUNK], F32, tag="r_b")
            scr = drp.tile([1, CHUNK], F32, tag="scr_r")
            nc.sync.dma_start(out=scr, in_=rr[:])
            nc.sync.dma_start(out=r_b, in_=scr.to_broadcast([P, CHUNK]))
            xsc = xscp.tile([P, KT_C, CHUNK], BF16, tag="xsc")
            nc.vector.tensor_tensor(
                xsc, xc, r_b[:, None, :].to_broadcast([P, KT_C, CHUNK]), OP.mult)
            aug = xscp.tile([P, CHUNK], BF16, tag="aug")
            nc.vector.memset(aug, 0.0)
            nc.sync.dma_start(out=aug[1:2], in_=ones_row[:])
            nc.vector.tensor_copy(out=aug[0:1], in_=murb)

            rhs5 = [xsc[:, kt] for kt in range(KT_C)] + [aug[:]]
            # QKV (Q/K only): 4 j-chunks of 128
            for jc in range(4):
                pq = ps_big.tile([P, CHUNK], F32, tag="big")
                for kt in range(KT_C + 1):
                    nc.tensor.matmul(pq, wt_sb[:, kt, jc * P:(jc + 1) * P],
                                     rhs5[kt],
                                     start=(kt == 0), stop=(kt == KT_C))
                nc.vector.tensor_copy(out=qk_tiles[jc][:, cs:cs + CHUNK], in_=pq)

            # V-direct: out (pos,260) per 128-pos subchunk
            for pc in range(CHUNK // P):
                pv = ps_small.tile([P, 260], F32, tag="small")
                for kt in range(KT_C + 1):
                    lhs = rhs5[kt][:, pc * P:(pc + 1) * P] if kt < KT_C else \
                        aug[:, pc * P:(pc + 1) * P]
                    nc.tensor.matmul(pv, lhs, wv_sb[:, kt, :],
                                     start=(kt == 0), stop=(kt == KT_C))
                kti = ch * (CHUNK // P) + pc
                nc.vector.tensor_copy(out=v_tiles[0][:, kti, :], in_=pv[:, 0:130])
                nc.vector.tensor_copy(out=v_tiles[1][:, kti, :], in_=pv[:, 130:260])

        # ================= Phase D: attention + proj =================
        for qc in range(nqc):
            qs = qc * CHUNK
            on_tiles = []
            for pr in range(NPAIR):
                q_t, k_t, v_t = qk_tiles[2 * pr], qk_tiles[2 * pr + 1], v_tiles[pr]
                po = [ps_small.tile([65, CHUNK], F32, tag="small", name=f"po{h2}")
                      for h2 in range(2)]
                for ktp in range(nkt // 2):
                    ps = [ps_big.tile([P, 2 * CHUNK], F32, tag="big", name="sps")
                          for _ in range(2)]
                    for h2 in range(2):
                        hsl = slice(h2 * D, (h2 + 1) * D)
                        for j in range(2):
                            kt = 2 * ktp + j
                            nc.tensor.matmul(
                                ps[h2][:, j * CHUNK:(j + 1) * CHUNK],
                                k_t[hsl, kt * P:(kt + 1) * P],
                                q_t[hsl, qs:qs + CHUNK],
                                start=True, stop=True)
                    et = [epool.tile([P, 2 * CHUNK], BF16, tag="et", name="et")
                          for _ in range(2)]
                    for h2 in range(2):
                        nc.scalar.activation(et[h2], ps[h2], AF.Exp, scale=LN2)
                    for j in range(2):
                        kt = 2 * ktp + j
                        for h2 in range(2):
                            nc.tensor.matmul(
                                po[h2], v_t[:, kt, h2 * 65:(h2 + 1) * 65],
                                et[h2][:, j * CHUNK:(j + 1) * CHUNK],
                                start=(ktp == 0 and j == 0),
                                stop=(ktp == nkt // 2 - 1 and j == 1))
                for h2 in range(2):
                    rden = rdp.tile([65, CHUNK], F32, tag="rden")
                    nc.vector.reciprocal(out=rden[64:65], in_=po[h2][64:65])
                    rb = rdp.tile([D, CHUNK], F32, tag="rb")
                    scr2 = drp.tile([1, CHUNK], F32, tag="scr_d")
                    nc.sync.dma_start(out=scr2, in_=rden[64:65])
                    nc.sync.dma_start(out=rb, in_=scr2.to_broadcast([D, CHUNK]))
                    on_t = onp.tile([D, CHUNK], BF16, tag="on")
                    nc.vector.tensor_tensor(on_t, po[h2][0:D], rb, OP.mult)
                    on_tiles.append(on_t)
            # proj: out^T (o, q) += sum_h Wp_h^T On_h
            for oc in range(C // P):
                pp = ps_small.tile([P, CHUNK], F32, tag="small")
                for hh in range(NHEADS_CORE):
                    nc.tensor.matmul(pp, wp_sb[:, hh, oc * P:(oc + 1) * P],
                                     on_tiles[hh],
                                     start=(hh == 0), stop=(hh == NHEADS_CORE - 1))
                outt = outp.tile([P, CHUNK], F32, tag="outt")
                nc.vector.tensor_scalar_add(outt, pp, bp_sb[:, oc:oc + 1])
                nc.sync.dma_start(out=out[oc * P:(oc + 1) * P, qs:qs + CHUNK],
                                  in_=outt)

    if hoist:
        _hoist_excess_waits(nc)
    return nc


# ---------------------------------------------------------------------------
# Host-side input prep

def _prep_core_inputs(X, gamma, beta, Wqkv, bqkv, Wproj, bproj):
    """Build the 8 per-core input maps (numpy)."""
    s = LOG2E / SM_SCALE
    f32 = np.float32
    Wg = (Wqkv * gamma[None, :]).astype(f32)            # (3C, C)
    gvec = (Wqkv @ gamma).astype(f32)                   # (3C,)
    c0 = (Wqkv @ beta + bqkv).astype(f32)               # (3C,)
    # full augmented row block: A_full[j] = [Wg[j], g[j], c0[j]]  (3C, C+2)
    A_full = np.concatenate([Wg, gvec[:, None], c0[:, None]], axis=1)

    def jidx(h, part):      # original row indices for head h, part 0/1/2=q/k/v
        return np.arange(h * (3 * D) + part * D, h * (3 * D) + (part + 1) * D)

    in_maps = []
    for c in range(8):
        b, hh = c // 2, c % 2
        heads = [4 * hh + i for i in range(4)]
        # ---- wt: Q/K lhsT (5,128,512) ----
        cols = np.zeros((C + 2, 512), dtype=f32)
        for jc, (part, h0, h1) in enumerate(
                [(0, 0, 1), (1, 0, 1), (0, 2, 3), (1, 2, 3)]):
            scale = s if part == 0 else 1.0
            cols[:, jc * P + 0:jc * P + D] = A_full[jidx(heads[h0], part)].T * scale
            cols[:, jc * P + D:jc * P + 2 * D] = A_full[jidx(heads[h1], part)].T * scale
        wt_np = np.zeros((5 * P, 512), dtype=f32)
        wt_np[:C + 2] = cols
        wt_np = wt_np.reshape(5, P, 512).astype(ml_dtypes.bfloat16)
        # ---- wv: V rhs (5,128,260) ----
        vcols = np.zeros((C + 2, 260), dtype=f32)
        for pr in range(2):
            for i2 in range(2):
                h = heads[2 * pr + i2]
                u0 = pr * 130 + i2 * 65
                vcols[:, u0:u0 + D] = A_full[jidx(h, 2)].T
                vcols[C + 1, u0 + D] = 1.0          # ones column
        wv_np = np.zeros((5 * P, 260), dtype=f32)
        wv_np[:C + 2] = vcols
        wv_np = wv_np.reshape(5, P, 260).astype(ml_dtypes.bfloat16)
        # ---- wp: (4, 64, 512) ----
        wp_np = np.zeros((NHEADS_CORE, D, C), dtype=f32)
        for i, h in enumerate(heads):
            wp_np[i] = Wproj[:, h * D:(h + 1) * D].T
        wp_np = wp_np.astype(ml_dtypes.bfloat16)
        bp_np = bproj.astype(f32) if hh == 0 else np.zeros(C, dtype=f32)
        x_np = np.ascontiguousarray(X[b].reshape(C, H * W)).astype(f32)
        in_maps.append({"x": x_np, "wt": np.ascontiguousarray(wt_np),
                        "wv": np.ascontiguousarray(wv_np),
                        "wp": np.ascontiguousarray(wp_np), "bp": bp_np})
    return in_maps


_NC_CACHE = {}


def _get_nc():
    if "nc" not in _NC_CACHE:
        _NC_CACHE["nc"] = build_core_program()
    return _NC_CACHE["nc"]


def kernel(X, gamma, beta, Wqkv, bqkv, Wproj, bproj):
    from concourse.bass_utils import run_bass_kernel_spmd

    X = np.asarray(X, dtype=np.float32)
    gamma = np.asarray(gamma, dtype=np.float32)
    beta = np.asarray(beta, dtype=np.float32)
    Wqkv = np.asarray(Wqkv, dtype=np.float32)
    bqkv = np.asarray(bqkv, dtype=np.float32)
    Wproj = np.asarray(Wproj, dtype=np.float32)
    bproj = np.asarray(bproj, dtype=np.float32)

    in_maps = _prep_core_inputs(X, gamma, beta, Wqkv, bqkv, Wproj, bproj)
    nc = _get_nc()
    res = run_bass_kernel_spmd(nc, in_maps, core_ids=list(range(8)))
    Y = np.empty((B, C, H, W), dtype=np.float32)
    for b in range(B):
        part = res.results[2 * b]["out"] + res.results[2 * b + 1]["out"]
        Y[b] = part.reshape(C, H, W)
    return Y
